# revision 1
# baseline (speedup 1.0000x reference)
"""2-layer GCN + edge-logit decoder on 8 Trainium2 NeuronCores.

Math (per layer, derived from PyG GCNConv with self-loops):
    dis = rsqrt(deg + 1)
    hn  = (x @ W) * dis[:, None]
    out[d] = dis[d] * (sum_{e: dst[e]=d} hn[src[e]] + hn[d]) + b
so the per-edge work is a pure gather(hn, src) -> scatter_add(agg, dst).

Sharding: nodes are sharded into 8 contiguous ranges of S=12544 (dst
ownership).  Edge lists are partitioned by dst owner on the host, and
sorted into 16 subgroups keyed by (src >> CH_SHIFT, src & 3):
  - the src>>CH_SHIFT chunk keeps gather indices within int16 range
  - the src&3 group lets layer-2 (16-float rows) gather 256-byte
    elements from a strided view with base offset (src&3)*16 floats
Each core computes the *full* hn1 table redundantly (cheaper than an
AllGather of 25.6MB).  hn2 and z2 shards are AllGathered.  The final
1M edge dot-products are sharded evenly; results are unpermuted on the
host.
"""

import math
import sys

import numpy as np

for _p in ("/opt/trn_rl_repo",):
    if _p not in sys.path:
        sys.path.append(_p)

import concourse.bacc as bacc
import concourse.bass as bass
import concourse.mybir as mybir
import concourse.tile as tile
from concourse import bass_utils
from concourse.masks import make_identity

F32 = mybir.dt.float32
I16 = mybir.dt.int16
AF = mybir.ActivationFunctionType
ALU = mybir.AluOpType


def default_cfg():
    return dict(
        N=100000,
        E=3200000,
        PAIRS=1000000,
        FEAT=128,
        HID=64,
        OUT=16,
        C=8,
        CH_SHIFT=15,  # 4 chunks of 2^15 node ids (int16 gather range)
        GCAP=6400,  # max edges per dma_gather instruction
        SCAP=6400,  # max edges per dma_scatter_add instruction
        TILE_F=7936,  # pairs per final gather instruction (62*128)
        XT_BLK=8,  # node tiles per xT DMA in the dense phase
        DMA_SCRATCH=16384,  # SWDGE descriptor-ring carveout (bytes/partition)
    )


def derive(cfg):
    d = dict(cfg)
    C = d["C"]
    d["S"] = int(math.ceil(d["N"] / C / 128)) * 128  # shard nodes / core
    d["NP"] = d["S"] * C  # padded node count
    d["G"] = d["NP"] // 128  # global node tiles
    d["GL"] = d["S"] // 128  # local node tiles
    d["CHUNK"] = 1 << d["CH_SHIFT"]
    assert 4 * d["CHUNK"] >= d["NP"] and d["CHUNK"] <= 32768
    d["NTAB1"] = 4 * d["CHUNK"]
    d["AGG_ROWS"] = d["S"] + 128  # + trash slots for padded edges
    d["M2"] = ((d["N"] - 1) >> 2) + 1  # packed-4 elements in 16-f tables
    assert d["M2"] <= 32768
    # flat f32 length of the 16-wide tables (hn2 / z2), covering the
    # strided 64-float gather views
    need = 3 * d["OUT"] + d["M2"] * d["HID"]
    d["NTAB2F"] = max(d["NP"] * d["OUT"], int(math.ceil(need / 2048)) * 2048)
    assert d["G"] % d["XT_BLK"] == 0
    return d


# ---------------------------------------------------------------- host prep


def _wrap16(arr):
    """[.., 16 subgroups, L] int16 -> [.., 16 partitions, 16*(L/16)] in the
    dma_gather index layout (idx i of subgroup s -> partition i%16, col
    s*(L/16) + i//16), then replicate to 128 partitions (8 Q7 cores)."""
    nsub, L = arr.shape[-2], arr.shape[-1]
    lead = arr.shape[:-2]
    a = arr.reshape(lead + (nsub, L // 16, 16))
    a = np.moveaxis(a, -1, -3)  # [..., 16, nsub, L//16]
    a = a.reshape(lead + (16, nsub * (L // 16)))
    return np.tile(a, (1,) * len(lead) + (8, 1)).astype(np.int16)


def _edge_plan(src, dstl, core_of, sub_of, idxval, d):
    """Occurrence-class edge layout for one layer.

    Edges are grouped per (core, subgroup) cell; within a cell they are
    ordered by the occurrence rank of their dst, and each rank class is
    padded to a uniform (over cells) multiple of 128.  Scatter chunks
    never cross class boundaries, so real dst values within one
    dma_scatter_add are unique (the HW CCE read-modify-write races on
    duplicates).  Pad edges gather row (i%128) of the view and scatter to
    write-off trash slots >= S where races are harmless.

    Returns (gidx[C,4,E_sub] i16, sidx[C,4,E_sub] i16, plan, E_sub) with
    plan = [(toff, tsize, [(soff_rel, ssize), ...]), ...] uniform across
    cells.
    """
    C, S, GCAP, SCAP = d["C"], d["S"], d["GCAP"], d["SCAP"]
    key = (core_of * 4 + sub_of).astype(np.int64)
    order = np.argsort(key, kind="stable")
    ks = key[order]
    bounds = np.searchsorted(ks, np.arange(C * 4 + 1))
    src_s = src[order]
    dstl_s = dstl[order]

    cells = []
    maxR = 1
    for cell in range(C * 4):
        b0, b1 = int(bounds[cell]), int(bounds[cell + 1])
        dseg = dstl_s[b0:b1]
        o2 = np.argsort(dseg, kind="stable")
        ds = dseg[o2]
        r_sorted = np.arange(ds.shape[0]) - np.searchsorted(ds, ds)
        r = np.empty_like(r_sorted)
        r[o2] = r_sorted
        cells.append((src_s[b0:b1], dseg, r))
        if ds.shape[0]:
            maxR = max(maxR, int(r_sorted.max()) + 1)

    Cr = []
    for rr in range(maxR):
        m = max(int((r == rr).sum()) for (_, _, r) in cells)
        Cr.append(max(128, int(math.ceil(m / 128)) * 128))
    E_sub = int(sum(Cr))

    gidx = np.empty((C, 4, E_sub), np.int16)
    sidx = np.empty((C, 4, E_sub), np.int16)
    for cell in range(C * 4):
        c, s = divmod(cell, 4)
        sv, dv, r = cells[cell]
        ro = np.argsort(r, kind="stable")
        rs = r[ro]
        off = 0
        for rr in range(maxR):
            a, b = np.searchsorted(rs, [rr, rr + 1])
            cnt = int(b - a)
            cap = Cr[rr]
            pad = np.arange(cap - cnt, dtype=np.int64) % 128
            sel = ro[a:b]
            gidx[c, s, off : off + cnt] = idxval(sv[sel], s)
            gidx[c, s, off + cnt : off + cap] = pad
            sidx[c, s, off : off + cnt] = dv[sel]
            sidx[c, s, off + cnt : off + cap] = S + pad
            off += cap

    # gather tiles (<= GCAP, cuts at class bounds or GCAP-chunks of one class)
    cb = np.cumsum([0] + Cr)
    tiles = []
    cur0 = 0
    for rr in range(maxR):
        c0, c1 = int(cb[rr]), int(cb[rr + 1])
        size = c1 - c0
        if size > GCAP:
            if c0 > cur0:
                tiles.append((cur0, c0 - cur0))
            p = c0
            while p < c1:
                t = min(GCAP, c1 - p)
                tiles.append((p, t))
                p += t
            cur0 = c1
        elif c1 - cur0 > GCAP:
            if c0 > cur0:
                tiles.append((cur0, c0 - cur0))
            cur0 = c0
    if E_sub > cur0:
        tiles.append((cur0, E_sub - cur0))

    plan = []
    for toff, tsize in tiles:
        scats = []
        for rr in range(maxR):
            c0, c1 = int(cb[rr]), int(cb[rr + 1])
            a, b = max(c0, toff), min(c1, toff + tsize)
            p = a
            while p < b:
                t = min(SCAP, b - p)
                scats.append((p - toff, t))
                p += t
        plan.append((toff, tsize, tuple(scats)))
    return gidx, sidx, tuple(plan), E_sub


def prep_host(inputs, cfg):
    d = cfg
    N, C, S, NP = d["N"], d["C"], d["S"], d["NP"]
    FEAT, HID, OUT = d["FEAT"], d["HID"], d["OUT"]
    TILE_F = d["TILE_F"]
    CHUNK, CH_SHIFT = d["CHUNK"], d["CH_SHIFT"]

    x = np.asarray(inputs["x"], np.float32)
    ei = np.asarray(inputs["edge_index"], np.int64)
    pe = np.asarray(inputs["pos_edge_index"], np.int64)
    ne = np.asarray(inputs["neg_edge_index"], np.int64)
    W1 = np.asarray(inputs["W1"], np.float32)
    b1 = np.asarray(inputs["b1"], np.float32)
    W2 = np.asarray(inputs["W2"], np.float32)
    b2 = np.asarray(inputs["b2"], np.float32)

    src, dst = ei[0], ei[1]
    E = src.shape[0]

    xp = np.zeros((NP, FEAT), np.float32)
    xp[:N] = x
    xT = np.ascontiguousarray(xp.T)  # [128, NP]

    deg = np.bincount(dst, minlength=NP).astype(np.float32) + 1.0
    degp_g = np.ascontiguousarray(deg.reshape(d["G"], 128).T)  # [128, G]

    # ---- edges: partition by dst owner; per layer, 4 subgroups by src and
    # occurrence-class layout so every scatter touches each dst at most once
    core_of = dst // S
    dstl = dst - core_of * S
    g1, s1, plan1, E_sub1 = _edge_plan(
        src, dstl, core_of, src >> CH_SHIFT,
        lambda sv, s: sv & (CHUNK - 1), d,
    )
    g2, s2, plan2, E_sub2 = _edge_plan(
        src, dstl, core_of, src & 3,
        lambda sv, s: sv >> 2, d,
    )
    gidx1 = _wrap16(g1)  # [C, 128, 4*E_sub1//16]
    sidx1 = _wrap16(s1)
    gidx2 = _wrap16(g2)
    sidx2 = _wrap16(s2)

    # ---- final pairs
    pq = np.concatenate([pe, ne], axis=1)  # [2, PAIRS]
    P = pq.shape[1]
    assert P % C == 0
    PC = P // C
    a = pq[0].reshape(C, PC)
    b = pq[1].reshape(C, PC)
    fkey = (a & 3) * 4 + (b & 3)
    forder = np.argsort(fkey, axis=1, kind="stable")
    fks = np.take_along_axis(fkey, forder, axis=1)
    a_s = np.take_along_axis(a, forder, axis=1)
    b_s = np.take_along_axis(b, forder, axis=1)
    fbounds = np.stack(
        [np.searchsorted(fks[c], np.arange(17)) for c in range(C)]
    )  # [C, 17]
    fcounts = fbounds[:, 1:] - fbounds[:, :-1]
    n_ft = max(1, int(math.ceil(fcounts.max() / TILE_F)))
    F_sub = n_ft * TILE_F

    fA = np.empty((C, 16, F_sub), np.int16)
    fB = np.empty((C, 16, F_sub), np.int16)
    # mapping: device output lin position (per core) -> global pair index
    TJ = TILE_F // 128
    i = np.arange(F_sub)
    t_i = i // TILE_F
    r = i % TILE_F
    lin_i = t_i * TILE_F + (r % 128) * TJ + (r // 128)  # within a subgroup blk
    out_pos = np.empty((C, 16 * F_sub), np.int64)  # positions into lraw
    out_src = np.empty((C, 16 * F_sub), np.int64)  # global pair idx, -1 pad
    for c in range(C):
        for s in range(16):
            b0, b1_ = fbounds[c, s], fbounds[c, s + 1]
            cnt = b1_ - b0
            pad = np.arange(F_sub - cnt, dtype=np.int64) % 128
            fA[c, s, :cnt] = a_s[c, b0:b1_] >> 2
            fA[c, s, cnt:] = pad
            fB[c, s, :cnt] = b_s[c, b0:b1_] >> 2
            fB[c, s, cnt:] = pad
            base = s * F_sub
            out_pos[c, base : base + F_sub] = s * n_ft * TILE_F + lin_i
            osrc = np.full(F_sub, -1, np.int64)
            osrc[:cnt] = c * PC + forder[c, b0:b1_]
            out_src[c, base : base + F_sub] = osrc
    fidxA = _wrap16(fA)
    fidxB = _wrap16(fB)

    in_maps = []
    for c in range(C):
        m = dict(
            xT=xT,
            xlT=np.ascontiguousarray(xp[c * S : (c + 1) * S].T),
            degp_g=degp_g,
            degp_l=np.ascontiguousarray(deg[c * S : (c + 1) * S].reshape(d["GL"], 128).T),
            w1=W1,
            w2=W2,
            b1r=np.ascontiguousarray(np.tile(b1[None, :], (128, 1))),
            b2r=np.ascontiguousarray(np.tile(b2[None, :], (128, 1))),
            gidx1=np.ascontiguousarray(gidx1[c]),
            sidx1=np.ascontiguousarray(sidx1[c]),
            gidx2=np.ascontiguousarray(gidx2[c]),
            sidx2=np.ascontiguousarray(sidx2[c]),
            fidxA=np.ascontiguousarray(fidxA[c]),
            fidxB=np.ascontiguousarray(fidxB[c]),
        )
        in_maps.append(m)

    meta = dict(
        plan1=plan1, E_sub1=E_sub1, plan2=plan2, E_sub2=E_sub2,
        n_ft=n_ft, P=P, out_pos=out_pos, out_src=out_src,
    )
    return in_maps, meta


def assemble(out_maps, meta, cfg):
    P = meta["P"]
    logits = np.zeros(P, np.float32)
    for c in range(cfg["C"]):
        lraw = out_maps[c]["lraw"].reshape(-1)
        pos = meta["out_pos"][c]
        srcg = meta["out_src"][c]
        valid = srcg >= 0
        logits[srcg[valid]] = lraw[pos[valid]]
    return logits


# ---------------------------------------------------------------- device build


def build(cfg, meta, enable_asserts=False):
    d = cfg
    C = d["C"]
    FEAT, HID, OUT = d["FEAT"], d["HID"], d["OUT"]
    S, NP, G, GL = d["S"], d["NP"], d["G"], d["GL"]
    TILE_F = d["TILE_F"]
    CHUNK = d["CHUNK"]
    plan1, E_sub1 = meta["plan1"], meta["E_sub1"]
    plan2, E_sub2 = meta["plan2"], meta["E_sub2"]
    n_ft = meta["n_ft"]
    F_sub = n_ft * TILE_F
    TJ_F = TILE_F // 128
    XB = d["XT_BLK"]

    nc = bacc.Bacc(
        "TRN2",
        target_bir_lowering=False,
        debug=False,
        enable_asserts=enable_asserts,
        num_devices=C,
        dynamic_dma_scratch_size=d["DMA_SCRATCH"],
    )

    # I/O
    xT = nc.dram_tensor("xT", [128, NP], F32, kind="ExternalInput")
    xlT = nc.dram_tensor("xlT", [128, S], F32, kind="ExternalInput")
    degp_g = nc.dram_tensor("degp_g", [128, G], F32, kind="ExternalInput")
    degp_l = nc.dram_tensor("degp_l", [128, GL], F32, kind="ExternalInput")
    w1 = nc.dram_tensor("w1", [FEAT, HID], F32, kind="ExternalInput")
    w2 = nc.dram_tensor("w2", [HID, OUT], F32, kind="ExternalInput")
    b1r = nc.dram_tensor("b1r", [128, HID], F32, kind="ExternalInput")
    b2r = nc.dram_tensor("b2r", [128, OUT], F32, kind="ExternalInput")
    gidx1 = nc.dram_tensor("gidx1", [128, E_sub1 * 4 // 16], I16, kind="ExternalInput")
    sidx1 = nc.dram_tensor("sidx1", [128, E_sub1 * 4 // 16], I16, kind="ExternalInput")
    gidx2 = nc.dram_tensor("gidx2", [128, E_sub2 * 4 // 16], I16, kind="ExternalInput")
    sidx2 = nc.dram_tensor("sidx2", [128, E_sub2 * 4 // 16], I16, kind="ExternalInput")
    fidxA = nc.dram_tensor("fidxA", [128, F_sub * 16 // 16], I16, kind="ExternalInput")
    fidxB = nc.dram_tensor("fidxB", [128, F_sub * 16 // 16], I16, kind="ExternalInput")
    lraw = nc.dram_tensor("lraw", [16 * F_sub], F32, kind="ExternalOutput")

    # internal DRAM
    hn1_t = nc.dram_tensor("hn1_t", [d["NTAB1"], HID], F32)
    agg1_t = nc.dram_tensor("agg1_t", [d["AGG_ROWS"], HID], F32)
    agg2_t = nc.dram_tensor("agg2_t", [d["AGG_ROWS"], HID], F32)
    hn2_sh = nc.dram_tensor("hn2_sh", [S * OUT], F32)
    z2_sh = nc.dram_tensor("z2_sh", [S * OUT], F32)
    hn2_t = nc.dram_tensor("hn2_t", [d["NTAB2F"]], F32, addr_space="Shared")
    z2_t = nc.dram_tensor("z2_t", [d["NTAB2F"]], F32, addr_space="Shared")

    groups = [list(range(C))]

    def tab2_view(t, par):
        # strided 64-float-element view of a 16-wide table, base (par*16)
        return t.ap()[par * OUT : par * OUT + d["M2"] * HID].rearrange(
            "(m e) -> m e", e=HID
        )

    with tile.TileContext(nc) as tc:
        with (
            tc.tile_pool(name="persist", bufs=1) as pP,
            tc.tile_pool(name="idx", bufs=4) as pIdx,
        ):
            # ---- persistent small tensors
            w1_sb = pP.tile([FEAT, HID], F32)
            nc.sync.dma_start(out=w1_sb[:], in_=w1[:, :])
            w2_sb = pP.tile([HID, OUT], F32)
            nc.sync.dma_start(out=w2_sb[:], in_=w2[:, :])
            b1_sb = pP.tile([128, HID], F32)
            nc.sync.dma_start(out=b1_sb[:], in_=b1r[:, :])
            b2_sb = pP.tile([128, OUT], F32)
            nc.sync.dma_start(out=b2_sb[:], in_=b2r[:, :])
            ident = pP.tile([128, 128], F32)
            make_identity(nc, ident[:])

            dg_raw = pP.tile([128, G], F32)
            nc.sync.dma_start(out=dg_raw[:], in_=degp_g[:, :])
            dis_g = pP.tile([128, G], F32)
            nc.vector.reciprocal(dis_g[:], dg_raw[:])
            nc.scalar.activation(dis_g[:], dis_g[:], AF.Sqrt)

            dl_raw = pP.tile([128, GL], F32)
            nc.sync.dma_start(out=dl_raw[:], in_=degp_l[:, :])
            dis_l = pP.tile([128, GL], F32)
            nc.vector.reciprocal(dis_l[:], dl_raw[:])
            nc.scalar.activation(dis_l[:], dis_l[:], AF.Sqrt)

            hn2_sb = pP.tile([128, GL * OUT], F32)  # kept for phase F

            # ---- zero the accumulator tables and never-written table tails
            ZCOLS = 4096
            with tc.tile_pool(name="zero", bufs=1) as pZ:
                zsb = pZ.tile([128, ZCOLS], F32)
                nc.vector.memset(zsb[:], 0.0)

                def zero_flat(flat_ap, n_floats):
                    assert n_floats % 128 == 0
                    off = 0
                    while off < n_floats:
                        f = min(ZCOLS, (n_floats - off) // 128)
                        nc.sync.dma_start(
                            out=flat_ap[off : off + 128 * f].rearrange(
                                "(p f) -> p f", f=f
                            ),
                            in_=zsb[:, 0:f],
                        )
                        off += 128 * f

                zero_flat(agg1_t.ap().rearrange("a b -> (a b)"), d["AGG_ROWS"] * HID)
                zero_flat(agg2_t.ap().rearrange("a b -> (a b)"), d["AGG_ROWS"] * HID)
                if d["NTAB1"] > NP:
                    zero_flat(
                        hn1_t.ap().rearrange("a b -> (a b)")[NP * HID :],
                        (d["NTAB1"] - NP) * HID,
                    )
                if d["NTAB2F"] > NP * OUT:
                    for tab in (hn2_t, z2_t):
                        zero_flat(tab.ap()[NP * OUT :], d["NTAB2F"] - NP * OUT)

            # ---- phase A: full hn1 table (redundant on every core)
            hn1_r = hn1_t.ap()[0:NP, :].rearrange("(g t p) d -> g p t d", t=XB, p=128)
            with tc.tile_pool(name="stream", bufs=3) as pS, tc.tile_pool(
                name="psumA", bufs=4, space="PSUM"
            ) as psA:
                for blk in range(G // XB):
                    xt = pS.tile([128, XB * FEAT], F32, tag="xt")
                    nc.sync.dma_start(
                        out=xt[:], in_=xT[:, blk * XB * FEAT : (blk + 1) * XB * FEAT]
                    )
                    hn_sb = pS.tile([128, XB * HID], F32, tag="hn")
                    for t in range(XB):
                        ps = psA.tile([128, HID], F32)
                        nc.tensor.matmul(
                            ps[:],
                            lhsT=xt[:, t * 128 : (t + 1) * 128],
                            rhs=w1_sb[:],
                            start=True,
                            stop=True,
                        )
                        g = blk * XB + t
                        nc.vector.tensor_scalar_mul(
                            hn_sb[:, t * HID : (t + 1) * HID], ps[:], dis_g[:, g : g + 1]
                        )
                    nc.sync.dma_start(
                        out=hn1_r[blk], in_=hn_sb[:].rearrange("p (t d) -> p t d", d=HID)
                    )

            # ---- edge gather/scatter phases
            def edge_phase(gidx_in, sidx_in, view_of_sub, agg_out, plan, E_sub, pMsg):
                for s in range(4):
                    base16 = s * E_sub // 16
                    for toff, tsize, scats in plan:
                        t16 = toff // 16
                        gi = pIdx.tile([128, tsize // 16], I16, tag="gi")
                        nc.sync.dma_start(
                            out=gi[:],
                            in_=gidx_in[:, base16 + t16 : base16 + t16 + tsize // 16],
                        )
                        si = pIdx.tile([128, tsize // 16], I16, tag="si")
                        nc.sync.dma_start(
                            out=si[:],
                            in_=sidx_in[:, base16 + t16 : base16 + t16 + tsize // 16],
                        )
                        msg = pMsg.tile([128, tsize // 128, HID], F32, tag="msg")
                        nc.gpsimd.dma_gather(
                            msg[:], view_of_sub(s), gi[:], tsize, tsize, HID,
                            single_packet=tsize <= 1024,
                        )
                        for soff, ssize in scats:
                            nc.gpsimd.dma_scatter_add(
                                agg_out.ap()[:, :],
                                msg[:, soff // 128 : (soff + ssize) // 128, :],
                                si[:, soff // 16 : (soff + ssize) // 16],
                                ssize,
                                ssize,
                                HID,
                                single_packet=ssize <= 1024,
                            )

            pMsg_cm = tc.tile_pool(name="msg", bufs=3)
            pMsg = pMsg_cm.__enter__()
            edge_phase(
                gidx1,
                sidx1,
                lambda s: hn1_t.ap()[s * CHUNK : (s + 1) * CHUNK, :],
                agg1_t,
                plan1,
                E_sub1,
                pMsg,
            )

            # ---- phase C: z = relu(...), hn2 shard, AllGather
            with tc.tile_pool(name="phc", bufs=3) as pC, tc.tile_pool(
                name="psC", bufs=2, space="PSUM"
            ) as psC:
                for g in range(GL):
                    xlt = pC.tile([128, FEAT], F32, tag="xlt")
                    nc.sync.dma_start(
                        out=xlt[:], in_=xlT[:, g * 128 : (g + 1) * 128]
                    )
                    ps_h = psC.tile([128, HID], F32, tag="psh")
                    nc.tensor.matmul(
                        ps_h[:], lhsT=xlt[:], rhs=w1_sb[:], start=True, stop=True
                    )
                    agg_sb = pC.tile([128, HID], F32, tag="agg")
                    nc.sync.dma_start(
                        out=agg_sb[:], in_=agg1_t[g * 128 : (g + 1) * 128, :]
                    )
                    zt = pC.tile([128, HID], F32, tag="zt")
                    # zt = hn_local = h * dis
                    nc.vector.tensor_scalar_mul(zt[:], ps_h[:], dis_l[:, g : g + 1])
                    nc.vector.tensor_tensor(
                        out=zt[:], in0=zt[:], in1=agg_sb[:], op=ALU.add
                    )
                    nc.vector.tensor_scalar_mul(zt[:], zt[:], dis_l[:, g : g + 1])
                    nc.vector.tensor_tensor(
                        out=zt[:], in0=zt[:], in1=b1_sb[:], op=ALU.add
                    )
                    nc.scalar.activation(zt[:], zt[:], AF.Relu)
                    ps_zT = psC.tile([64, 128], F32, tag="pszt")
                    nc.tensor.transpose(ps_zT[:], zt[:], ident[:])
                    zT_sb = pC.tile([64, 128], F32, tag="ztT")
                    nc.vector.tensor_copy(zT_sb[:], ps_zT[:])
                    ps_h2 = psC.tile([128, OUT], F32, tag="psh2")
                    nc.tensor.matmul(
                        ps_h2[:], lhsT=zT_sb[:], rhs=w2_sb[:], start=True, stop=True
                    )
                    nc.vector.tensor_scalar_mul(
                        hn2_sb[:, g * OUT : (g + 1) * OUT],
                        ps_h2[:],
                        dis_l[:, g : g + 1],
                    )
                    nc.sync.dma_start(
                        out=hn2_sh.ap()[g * 128 * OUT : (g + 1) * 128 * OUT].rearrange(
                            "(p d) -> p d", d=OUT
                        ),
                        in_=hn2_sb[:, g * OUT : (g + 1) * OUT],
                    )

            nc.gpsimd.collective_compute(
                "AllGather",
                ALU.bypass,
                replica_groups=groups,
                ins=[hn2_sh.ap()],
                outs=[hn2_t.ap()[0 : NP * OUT]],
            )

            # ---- phase B2: layer-2 edge gather/scatter
            edge_phase(
                gidx2, sidx2, lambda s: tab2_view(hn2_t, s), agg2_t, plan2, E_sub2, pMsg
            )
            pMsg_cm.__exit__(None, None, None)

            # ---- phase F: z2 shard, AllGather
            with tc.tile_pool(name="phf", bufs=3) as pF:
                for g in range(GL):
                    agg_sb = pF.tile([128, OUT], F32, tag="agg2")
                    nc.sync.dma_start(
                        out=agg_sb[:], in_=agg2_t[g * 128 : (g + 1) * 128, 0:OUT]
                    )
                    z2 = pF.tile([128, OUT], F32, tag="z2")
                    nc.vector.tensor_tensor(
                        out=z2[:],
                        in0=agg_sb[:],
                        in1=hn2_sb[:, g * OUT : (g + 1) * OUT],
                        op=ALU.add,
                    )
                    nc.vector.tensor_scalar_mul(z2[:], z2[:], dis_l[:, g : g + 1])
                    nc.vector.tensor_tensor(
                        out=z2[:], in0=z2[:], in1=b2_sb[:], op=ALU.add
                    )
                    nc.sync.dma_start(
                        out=z2_sh.ap()[g * 128 * OUT : (g + 1) * 128 * OUT].rearrange(
                            "(p d) -> p d", d=OUT
                        ),
                        in_=z2[:],
                    )

            nc.gpsimd.collective_compute(
                "AllGather",
                ALU.bypass,
                replica_groups=groups,
                ins=[z2_sh.ap()],
                outs=[z2_t.ap()[0 : NP * OUT]],
            )

            # ---- final: edge logits
            with tc.tile_pool(name="fin", bufs=3) as pFin:
                colsF = TILE_F // 16
                for s in range(16):
                    for t in range(n_ft):
                        off16 = (s * n_ft + t) * colsF
                        fa = pIdx.tile([128, colsF], I16, tag="fa")
                        nc.sync.dma_start(
                            out=fa[:], in_=fidxA[:, off16 : off16 + colsF]
                        )
                        fb = pIdx.tile([128, colsF], I16, tag="fb")
                        nc.sync.dma_start(
                            out=fb[:], in_=fidxB[:, off16 : off16 + colsF]
                        )
                        ma = pFin.tile([128, TJ_F, HID], F32, tag="ma")
                        nc.gpsimd.dma_gather(
                            ma[:], tab2_view(z2_t, s >> 2), fa[:], TILE_F, TILE_F, HID,
                            single_packet=TILE_F <= 1024,
                        )
                        mb = pFin.tile([128, TJ_F, HID], F32, tag="mb")
                        nc.gpsimd.dma_gather(
                            mb[:], tab2_view(z2_t, s & 3), fb[:], TILE_F, TILE_F, HID,
                            single_packet=TILE_F <= 1024,
                        )
                        prod = pFin.tile([128, TJ_F, OUT], F32, tag="prod")
                        nc.vector.tensor_tensor(
                            out=prod[:],
                            in0=ma[:, :, 0:OUT],
                            in1=mb[:, :, 0:OUT],
                            op=ALU.mult,
                        )
                        red = pFin.tile([128, TJ_F], F32, tag="red")
                        nc.vector.reduce_sum(
                            out=red[:, :, None],
                            in_=prod[:],
                            axis=mybir.AxisListType.X,
                        )
                        blk = s * n_ft + t
                        nc.sync.dma_start(
                            out=lraw.ap()[
                                blk * TILE_F : (blk + 1) * TILE_F
                            ].rearrange("(p j) -> p j", j=TJ_F),
                            in_=red[:],
                        )

    nc.compile()
    return nc


# ---------------------------------------------------------------- entry point

_CACHE = {}
TRACE = False
LAST = {}


def kernel(**inputs):
    cfg = derive(default_cfg())
    in_maps, meta = prep_host(inputs, cfg)
    key = (meta["plan1"], meta["plan2"], meta["n_ft"])
    if key not in _CACHE:
        _CACHE[key] = build(cfg, meta)
    nc = _CACHE[key]
    res = bass_utils.run_bass_kernel_spmd(
        nc, in_maps, core_ids=list(range(cfg["C"])), trace=TRACE
    )
    LAST["res"] = res
    return assemble(res.results, meta, cfg)

